# revision 11
# baseline (speedup 1.0000x reference)
"""Causal multi-head attention (B=1, S=4096, D=768, H=12, d_head=64) on 8
Trainium2 NeuronCores.

Sharding: tensor-parallel over heads. 12 heads are mapped onto 16 head-slots
(2 per core); the 4 leftover heads are duplicated onto two slots of the same
core with their W_out rows pre-scaled by 0.5, keeping the SPMD program
uniform across cores. The host sums the 8 partial outputs and adds b_out
(the all-reduce step of the row-parallel out projection).

v4 (from v3):
- x is pre-transposed on the host (xT [D, S]); the on-device transpose
  pipeline (192 PE transposes at transpose-mode half clock + DVE repacks)
  is gone. Projections consume xT chunks directly.
- Score matmuls are K=64 row-tiled: slot 0 lives in array rows 0-63,
  slot 1 in rows 64-127 (tile_position auto-derived from base partitions),
  so the two slots' score matmuls run concurrently -> ~2x on scores.
- Normalize chain off the ACT queue: DVE drains ctx PSUM (was scalar.copy
  stuck behind the exp backlog -> ~5us PE stall per tile), denominator
  reciprocal via reciprocal_approx_fast (5x faster than reciprocal).
- Output partials in fp16 (halves output DMA, host accumulates in fp32).
"""

import sys

sys.path.insert(0, "/opt/trn_rl_repo")

import ml_dtypes
import numpy as np

import concourse.bass as bass
import concourse.tile as tile
from concourse import bacc, mybir
from concourse.bass_utils import run_bass_kernel_spmd

S = 4096
D = 768
HD = 64
P = 128
KC = D // P  # 6 contraction chunks for the projections
QT_W = 512  # query-tile width (psum free dim)
NQT = S // QT_W  # 8 query tiles
NKB = S // P  # 32 key blocks
NEG = -1e30

F32 = mybir.dt.float32
F32R = mybir.dt.float32r
F16 = mybir.dt.float16
BF16 = mybir.dt.bfloat16
AF = mybir.ActivationFunctionType
ALU = mybir.AluOpType

SLOTS = [(0, 1), (2, 3), (4, 5), (6, 7), (8, 8), (9, 9), (10, 10), (11, 11)]
SCALES = [(1.0, 1.0)] * 4 + [(0.5, 0.5)] * 4

_CACHED_NC = None


def build_nc(dbg=False):
    nc = bacc.Bacc("TRN2", target_bir_lowering=False, debug=False, num_devices=8)

    if dbg:
        qT_dd = nc.declare_dram_parameter("qT_dbg", [P, S], BF16, isOutput=True)
        kT_dd = nc.declare_dram_parameter("kT_dbg", [P, S], BF16, isOutput=True)
        vA_dd = nc.declare_dram_parameter(
            "vA_dbg", [P, NKB, 2, HD + 1], BF16, isOutput=True
        )
        cT_dd = nc.declare_dram_parameter("cT_dbg", [P, S], BF16, isOutput=True)

    xT_d = nc.declare_dram_parameter("xT", [D, S], BF16, isOutput=False)
    w_d = nc.declare_dram_parameter("w", [D, 3 * P], BF16, isOutput=False)
    wo_d = nc.declare_dram_parameter("wo", [P, D], F32, isOutput=False)
    mask_d = nc.declare_dram_parameter("mask", [P, P], F32, isOutput=False)
    ident_d = nc.declare_dram_parameter("ident", [P, P], BF16, isOutput=False)
    out_d = nc.declare_dram_parameter("out", [S, D], F16, isOutput=True)

    with tile.TileContext(nc) as tc:
        with (
            tc.tile_pool(name="const", bufs=1) as const,
            tc.tile_pool(name="big", bufs=1) as big,
        ):
            # ---- constants ----
            mask3 = const.tile([P, 1, P], F32)
            nc.sync.dma_start(mask3[:, 0, :], mask_d[:])
            ident = const.tile([P, P], BF16)
            nc.sync.dma_start(ident[:], ident_d[:])
            ones_c = const.tile([P, 1], BF16)
            nc.gpsimd.memset(ones_c[:], 1.0)
            w_r = const.tile([P, KC, 3 * P], BF16)
            nc.sync.dma_start(w_r[:], w_d.rearrange("(c p) m -> p c m", p=P))
            wo_r = const.tile([P, D], BF16)
            with tc.tile_pool(name="wst", bufs=1) as wst:
                wo_stage = wst.tile([P, D], F32)
                nc.sync.dma_start(wo_stage[:], wo_d[:])
                nc.vector.tensor_copy(wo_r[:], wo_stage[:])

            # Q^T / K^T with slot s in partitions [64s, 64s+64); V natural
            # [keys, slot, 65] with a ones column at 64 for the softmax
            # denominator.
            qT = big.tile([P, S], BF16)
            k2 = big.tile([P, S], BF16)
            vA = big.tile([P, NKB, 2, HD + 1], BF16)
            cT = big.tile([P, S], BF16)
            for slot in (0, 1):
                nc.vector.tensor_copy(
                    vA[:, :, slot, HD], ones_c[:, 0:1].broadcast_to([P, NKB])
                )

            xT_r = xT_d.rearrange("(c p) s -> p c s", p=P)

            with (
                tc.tile_pool(name="xs", bufs=3) as xs,
                tc.tile_pool(name="aux", bufs=2, space="PSUM") as aux,
                tc.tile_pool(name="work", bufs=2, space="PSUM") as work,
                tc.tile_pool(name="ctxp", bufs=1, space="PSUM") as ctxp,
                tc.tile_pool(name="sm", bufs=4) as sm,
                tc.tile_pool(name="pt", bufs=4) as pt,
            ):
                # warmup matmuls: ramp the PE HAM toward 2.4 GHz while the
                # first DMAs stream in (real matmuls - transpose-mode does
                # not count as PE-busy for the HAM)
                for wi in range(6):
                    wt = aux.tile([P, 4, P], F32, name="wt", tag="aux")
                    for c in range(4):
                        nc.tensor.matmul(
                            wt[:, c, :], ident[:], ident[:], start=True, stop=True
                        )

                stage_map = {}

                def dma_group(t):
                    xTt = xs.tile([P, KC, QT_W], BF16, name="xTt", tag="xs")
                    nc.sync.dma_start(
                        xTt[:], xT_r[:, :, t * QT_W : (t + 1) * QT_W]
                    )
                    stage_map[t] = xTt

                def phaseAB(t):
                    """Q/K/V projections for group t (xT already
                    DMA-prefetched), as an interleavable generator."""
                    xTt = stage_map.pop(t)
                    if t + 1 < NQT and t + 1 not in stage_map:
                        dma_group(t + 1)
                    yield
                    pq = aux.tile([P, QT_W], F32, name="pq", tag="aux")
                    for c in range(KC):
                        nc.tensor.matmul(
                            pq[:],
                            w_r[:, c, 0:P],
                            xTt[:, c, :],
                            start=(c == 0),
                            stop=(c == KC - 1),
                        )
                    yield
                    nc.vector.tensor_copy(qT[:, t * QT_W : (t + 1) * QT_W], pq[:])
                    pk = aux.tile([P, QT_W], F32, name="pk", tag="aux")
                    for c in range(KC):
                        nc.tensor.matmul(
                            pk[:],
                            w_r[:, c, P : 2 * P],
                            xTt[:, c, :],
                            start=(c == 0),
                            stop=(c == KC - 1),
                        )
                    yield
                    nc.vector.tensor_copy(k2[:, t * QT_W : (t + 1) * QT_W], pk[:])
                    pv = aux.tile([P, QT_W], F32, name="pv", tag="aux")
                    for c in range(KC):
                        nc.tensor.matmul(
                            pv[:],
                            w_r[:, c, 2 * P : 3 * P],
                            xTt[:, c, :],
                            start=(c == 0),
                            stop=(c == KC - 1),
                        )
                    yield
                    vt_t = sm.tile([P, QT_W], BF16, name="vt_t", tag="vt", bufs=2)
                    nc.vector.tensor_copy(vt_t[:], pv[:])
                    vp = aux.tile([P, 4, P], BF16, name="vp", tag="aux")
                    for b in range(4):
                        nc.tensor.transpose(
                            vp[:, b, :], vt_t[:, b * P : (b + 1) * P], ident[:]
                        )
                    yield
                    nc.vector.tensor_copy(
                        vA[:, t * 4 : (t + 1) * 4, :, 0:HD],
                        vp[:].rearrange("p b (s h) -> p b s h", s=2),
                    )
                    yield

                def outproj(st):
                    o_stage = sm.tile([P, D], F16, name="o_stage", tag="ost", bufs=3)
                    for nch in range(2):
                        po = aux.tile([P, D // 2], F32, name="po", tag="aux")
                        nc.tensor.matmul(
                            po[:],
                            cT[:, st * P : (st + 1) * P],
                            wo_r[:, nch * (D // 2) : (nch + 1) * (D // 2)],
                            start=True,
                            stop=True,
                        )
                        nc.vector.tensor_copy(
                            o_stage[:, nch * (D // 2) : (nch + 1) * (D // 2)], po[:]
                        )
                    nc.sync.dma_start(out_d[st * P : (st + 1) * P, :], o_stage[:])

                # prologue: stage group 0 fully, prefetch group 1's x
                dma_group(0)
                for _ in phaseAB(0):
                    pass

                for t in range(NQT):
                    # pieces to interleave into this tile's kb loop: phase A/B
                    # of tile t+1 first, then out-projections of tile t-1
                    pieces = []
                    if t > 0:
                        pieces += [("op", (t - 1) * 4 + i) for i in range(4)]
                    gen = phaseAB(t + 1) if t + 1 < NQT else None

                    def next_piece(allow_op=True):
                        nonlocal gen
                        if gen is not None:
                            try:
                                next(gen)
                                return True
                            except StopIteration:
                                gen = None
                        if pieces and allow_op:
                            kind, v = pieces.pop(0)
                            outproj(v)
                            return True
                        return False

                    ctx = ctxp.tile([HD + 1, 2, QT_W], F32, name="ctx", tag="c")
                    nkb = 4 * (t + 1)
                    for kb in range(nkb):
                        r = kb * P - t * QT_W  # diagonal offset
                        r0 = max(0, r)
                        sc = work.tile([P, 2, QT_W], F32, name="sc", tag="w")
                        # K=64 row-tiled: slot 0 in array rows 0-63, slot 1 in
                        # rows 64-127 -> the two matmuls run concurrently
                        for slot in (0, 1):
                            nc.tensor.matmul(
                                sc[:, slot, r0:QT_W],
                                k2[
                                    slot * HD : (slot + 1) * HD,
                                    kb * P : (kb + 1) * P,
                                ],
                                qT[
                                    slot * HD : (slot + 1) * HD,
                                    t * QT_W + r0 : (t + 1) * QT_W,
                                ],
                                start=True,
                                stop=True,
                            )
                        if r >= 0:
                            nc.vector.tensor_tensor(
                                sc[:, :, r : r + P],
                                sc[:, :, r : r + P],
                                mask3[:, 0:1, :].broadcast_to([P, 2, P]),
                                ALU.add,
                            )
                        p2 = pt.tile([P, 2, QT_W], BF16, name="p2", tag="p")
                        nc.scalar.activation(
                            p2[:, :, r0:QT_W],
                            sc[:, :, r0:QT_W],
                            AF.Exp,
                            scale=0.125,
                        )
                        for slot in (0, 1):
                            nc.tensor.matmul(
                                ctx[:, slot, r0:QT_W],
                                vA[:, kb, slot, :],
                                p2[:, slot, r0:QT_W],
                                start=(kb == 0),
                                stop=(kb == nkb - 1),
                            )
                        next_piece(allow_op=(kb >= 10))

                    # drain leftover pieces BEFORE the normalize chain: the
                    # next tile's first score matmuls depend on phaseAB(t+1)'s
                    # qT/k2 casts, and the DVE executes in program order — if
                    # those casts queue behind the normalize chain, the PE
                    # stalls ~5us at every tile boundary (and HAM re-throttles)
                    while next_piece():
                        pass

                    # ---- normalize into cT ----
                    # DVE drains the ctx PSUM banks (off the busy ACT queue);
                    # the reciprocal / broadcast / multiply chain then runs
                    # off the SBUF copy while the next tile's kb loop starts.
                    # The reciprocal is issued in small chunks so it never
                    # blocks the DVE FIFO for long (one [1,2,512] op is
                    # ~6.6us and stalls everything queued behind it).
                    ctx_sb = sm.tile(
                        [HD + 1, 2, QT_W], F32, name="ctx_sb", tag="csb", bufs=2
                    )
                    nc.vector.tensor_copy(ctx_sb[:], ctx[:])
                    lr = sm.tile([1, 2, QT_W], F32, name="lrecip", tag="lr")
                    RC = 8
                    for rc in range(RC):
                        w0, w1 = rc * (QT_W // RC), (rc + 1) * (QT_W // RC)
                        nc.vector.reciprocal(
                            lr[:, :, w0:w1], ctx_sb[HD : HD + 1, :, w0:w1]
                        )
                    lb = sm.tile([HD, 2, QT_W], F32, name="lb", tag="lb", bufs=2)
                    if t == NQT - 1:
                        # last tile: block-granular so each out-projection
                        # fires as soon as its 128-wide cT block is ready
                        for b in range(4):
                            nc.gpsimd.partition_broadcast(
                                lb[:, :, b * P : (b + 1) * P],
                                lr[0:1, :, b * P : (b + 1) * P],
                            )
                            for slot in (0, 1):
                                nc.vector.tensor_tensor(
                                    cT[
                                        slot * HD : (slot + 1) * HD,
                                        t * QT_W + b * P : t * QT_W + (b + 1) * P,
                                    ],
                                    ctx_sb[0:HD, slot, b * P : (b + 1) * P],
                                    lb[:, slot, b * P : (b + 1) * P],
                                    ALU.mult,
                                )
                            outproj(t * 4 + b)
                    else:
                        nc.gpsimd.partition_broadcast(lb[:], lr[0:1, :, :])
                        for slot in (0, 1):
                            nc.vector.tensor_tensor(
                                cT[
                                    slot * HD : (slot + 1) * HD,
                                    t * QT_W : (t + 1) * QT_W,
                                ],
                                ctx_sb[0:HD, slot, :],
                                lb[:, slot, :],
                                ALU.mult,
                            )

                if dbg:
                    nc.sync.dma_start(qT_dd[:], qT[:])
                    nc.sync.dma_start(kT_dd[:], k2[:])
                    nc.sync.dma_start(vA_dd[:], vA[:])
                    nc.sync.dma_start(cT_dd[:], cT[:])

    nc.compile()
    return nc


def _host_inputs(x, W_query, W_key, W_value, W_out):
    mask = np.where(
        np.arange(P)[:, None] <= np.arange(P)[None, :], 0.0, NEG
    ).astype(np.float32)
    ident = np.eye(P, dtype=ml_dtypes.bfloat16)
    xT_bf = np.ascontiguousarray(x.T.astype(ml_dtypes.bfloat16))
    in_maps = []
    for core in range(8):
        ha, hb = SLOTS[core]
        sa, sb = SCALES[core]
        ca, cb = slice(ha * HD, (ha + 1) * HD), slice(hb * HD, (hb + 1) * HD)
        w_all = np.concatenate(
            [
                W_query[:, ca],
                W_query[:, cb],
                W_key[:, ca],
                W_key[:, cb],
                W_value[:, ca],
                W_value[:, cb],
            ],
            axis=1,
        )
        in_maps.append(
            {
                "xT": xT_bf,
                "w": np.ascontiguousarray(w_all.astype(ml_dtypes.bfloat16)),
                "wo": np.ascontiguousarray(
                    np.concatenate([W_out[ca, :] * sa, W_out[cb, :] * sb], axis=0)
                ).astype(np.float32),
                "mask": mask,
                "ident": ident,
            }
        )
    return in_maps


def run(x, W_query, W_key, W_value, W_out, b_out, trace=False):
    global _CACHED_NC
    if _CACHED_NC is None:
        _CACHED_NC = build_nc()
    nc = _CACHED_NC
    in_maps = _host_inputs(x, W_query, W_key, W_value, W_out)
    res = run_bass_kernel_spmd(nc, in_maps, core_ids=list(range(8)), trace=trace)
    out = np.zeros((S, D), dtype=np.float32)
    for core in range(8):
        out += res.results[core]["out"].astype(np.float32)
    out += b_out[None, :].astype(np.float32)
    return out, res


def kernel(x, W_query, W_key, W_value, W_out, b_out):
    x2 = np.asarray(x, dtype=np.float32).reshape(S, D)
    out, _ = run(
        x2,
        np.asarray(W_query, np.float32),
        np.asarray(W_key, np.float32),
        np.asarray(W_value, np.float32),
        np.asarray(W_out, np.float32),
        np.asarray(b_out, np.float32),
    )
    return out.reshape(1, S, D)


# revision 12
# speedup vs baseline: 1.0048x; 1.0048x over previous
"""Causal multi-head attention (B=1, S=4096, D=768, H=12, d_head=64) on 8
Trainium2 NeuronCores.

Sharding: tensor-parallel over heads. 12 heads are mapped onto 16 head-slots
(2 per core); the 4 leftover heads are duplicated onto two slots of the same
core with their W_out rows pre-scaled by 0.5, keeping the SPMD program
uniform across cores. The host sums the 8 partial outputs and adds b_out
(the all-reduce step of the row-parallel out projection).

v4 (from v3):
- x is pre-transposed on the host (xT [D, S]); the on-device transpose
  pipeline (192 PE transposes at transpose-mode half clock + DVE repacks)
  is gone. Projections consume xT chunks directly.
- Score matmuls are K=64 row-tiled: slot 0 lives in array rows 0-63,
  slot 1 in rows 64-127 (tile_position auto-derived from base partitions),
  so the two slots' score matmuls run concurrently -> ~2x on scores.
- Normalize chain off the ACT queue: DVE drains ctx PSUM (was scalar.copy
  stuck behind the exp backlog -> ~5us PE stall per tile), denominator
  reciprocal via reciprocal_approx_fast (5x faster than reciprocal).
- Output partials in fp16 (halves output DMA, host accumulates in fp32).
"""

import sys

sys.path.insert(0, "/opt/trn_rl_repo")

import ml_dtypes
import numpy as np

import concourse.bass as bass
import concourse.tile as tile
from concourse import bacc, mybir
from concourse.bass_utils import run_bass_kernel_spmd

S = 4096
D = 768
HD = 64
P = 128
KC = D // P  # 6 contraction chunks for the projections
QT_W = 512  # query-tile width (psum free dim)
NQT = S // QT_W  # 8 query tiles
NKB = S // P  # 32 key blocks
NEG = -1e30

F32 = mybir.dt.float32
F32R = mybir.dt.float32r
F16 = mybir.dt.float16
BF16 = mybir.dt.bfloat16
AF = mybir.ActivationFunctionType
ALU = mybir.AluOpType

SLOTS = [(0, 1), (2, 3), (4, 5), (6, 7), (8, 8), (9, 9), (10, 10), (11, 11)]
SCALES = [(1.0, 1.0)] * 4 + [(0.5, 0.5)] * 4

_CACHED_NC = None


def build_nc(dbg=False):
    nc = bacc.Bacc("TRN2", target_bir_lowering=False, debug=False, num_devices=8)

    if dbg:
        qT_dd = nc.declare_dram_parameter("qT_dbg", [P, S], BF16, isOutput=True)
        kT_dd = nc.declare_dram_parameter("kT_dbg", [P, S], BF16, isOutput=True)
        vA_dd = nc.declare_dram_parameter(
            "vA_dbg", [P, NKB, 2, HD + 1], BF16, isOutput=True
        )
        cT_dd = nc.declare_dram_parameter("cT_dbg", [P, S], BF16, isOutput=True)

    xT_d = nc.declare_dram_parameter("xT", [D, S], BF16, isOutput=False)
    w_d = nc.declare_dram_parameter("w", [D, 3 * P], BF16, isOutput=False)
    wo_d = nc.declare_dram_parameter("wo", [P, D], F32, isOutput=False)
    mask_d = nc.declare_dram_parameter("mask", [P, P], F32, isOutput=False)
    ident_d = nc.declare_dram_parameter("ident", [P, P], BF16, isOutput=False)
    out_d = nc.declare_dram_parameter("out", [S, D], F16, isOutput=True)

    with tile.TileContext(nc) as tc:
        with (
            tc.tile_pool(name="const", bufs=1) as const,
            tc.tile_pool(name="big", bufs=1) as big,
        ):
            # ---- constants ----
            mask3 = const.tile([P, 1, P], F32)
            nc.sync.dma_start(mask3[:, 0, :], mask_d[:])
            ident = const.tile([P, P], BF16)
            nc.sync.dma_start(ident[:], ident_d[:])
            ones_c = const.tile([P, 1], BF16)
            nc.gpsimd.memset(ones_c[:], 1.0)
            w_r = const.tile([P, KC, 3 * P], BF16)
            nc.sync.dma_start(w_r[:], w_d.rearrange("(c p) m -> p c m", p=P))
            wo_r = const.tile([P, D], BF16)
            with tc.tile_pool(name="wst", bufs=1) as wst:
                wo_stage = wst.tile([P, D], F32)
                nc.sync.dma_start(wo_stage[:], wo_d[:])
                nc.vector.tensor_copy(wo_r[:], wo_stage[:])

            # Q^T / K^T with slot s in partitions [64s, 64s+64); V natural
            # [keys, slot, 65] with a ones column at 64 for the softmax
            # denominator.
            qT = big.tile([P, S], BF16)
            k2 = big.tile([P, S], BF16)
            vA = big.tile([P, NKB, 2, HD + 1], BF16)
            cT = big.tile([P, S], BF16)
            for slot in (0, 1):
                nc.vector.tensor_copy(
                    vA[:, :, slot, HD], ones_c[:, 0:1].broadcast_to([P, NKB])
                )

            xT_r = xT_d.rearrange("(c p) s -> p c s", p=P)

            with (
                tc.tile_pool(name="xs", bufs=3) as xs,
                tc.tile_pool(name="aux", bufs=2, space="PSUM") as aux,
                tc.tile_pool(name="work", bufs=2, space="PSUM") as work,
                tc.tile_pool(name="ctxp", bufs=1, space="PSUM") as ctxp,
                tc.tile_pool(name="sm", bufs=4) as sm,
                tc.tile_pool(name="pt", bufs=4) as pt,
            ):
                # warmup matmuls: ramp the PE HAM toward 2.4 GHz while the
                # first DMAs stream in (real matmuls - transpose-mode does
                # not count as PE-busy for the HAM)
                for wi in range(6):
                    wt = aux.tile([P, 4, P], F32, name="wt", tag="aux")
                    for c in range(4):
                        nc.tensor.matmul(
                            wt[:, c, :], ident[:], ident[:], start=True, stop=True
                        )

                stage_map = {}

                def dma_group(t):
                    xTt = xs.tile([P, KC, QT_W], BF16, name="xTt", tag="xs")
                    nc.sync.dma_start(
                        xTt[:], xT_r[:, :, t * QT_W : (t + 1) * QT_W]
                    )
                    stage_map[t] = xTt

                def phaseAB(t):
                    """Q/K/V projections for group t (xT already
                    DMA-prefetched), as an interleavable generator."""
                    xTt = stage_map.pop(t)
                    if t + 1 < NQT and t + 1 not in stage_map:
                        dma_group(t + 1)
                    yield
                    pq = aux.tile([P, QT_W], F32, name="pq", tag="aux")
                    for c in range(KC):
                        nc.tensor.matmul(
                            pq[:],
                            w_r[:, c, 0:P],
                            xTt[:, c, :],
                            start=(c == 0),
                            stop=(c == KC - 1),
                        )
                    yield
                    nc.vector.tensor_copy(qT[:, t * QT_W : (t + 1) * QT_W], pq[:])
                    pk = aux.tile([P, QT_W], F32, name="pk", tag="aux")
                    for c in range(KC):
                        nc.tensor.matmul(
                            pk[:],
                            w_r[:, c, P : 2 * P],
                            xTt[:, c, :],
                            start=(c == 0),
                            stop=(c == KC - 1),
                        )
                    yield
                    nc.vector.tensor_copy(k2[:, t * QT_W : (t + 1) * QT_W], pk[:])
                    pv = aux.tile([P, QT_W], F32, name="pv", tag="aux")
                    for c in range(KC):
                        nc.tensor.matmul(
                            pv[:],
                            w_r[:, c, 2 * P : 3 * P],
                            xTt[:, c, :],
                            start=(c == 0),
                            stop=(c == KC - 1),
                        )
                    yield
                    vt_t = sm.tile([P, QT_W], BF16, name="vt_t", tag="vt", bufs=2)
                    nc.vector.tensor_copy(vt_t[:], pv[:])
                    vp = aux.tile([P, 4, P], BF16, name="vp", tag="aux")
                    for b in range(4):
                        nc.tensor.transpose(
                            vp[:, b, :], vt_t[:, b * P : (b + 1) * P], ident[:]
                        )
                    yield
                    nc.vector.tensor_copy(
                        vA[:, t * 4 : (t + 1) * 4, :, 0:HD],
                        vp[:].rearrange("p b (s h) -> p b s h", s=2),
                    )
                    yield

                def outproj(st):
                    o_stage = sm.tile([P, D], F16, name="o_stage", tag="ost", bufs=3)
                    for nch in range(2):
                        po = aux.tile([P, D // 2], F32, name="po", tag="aux")
                        nc.tensor.matmul(
                            po[:],
                            cT[:, st * P : (st + 1) * P],
                            wo_r[:, nch * (D // 2) : (nch + 1) * (D // 2)],
                            start=True,
                            stop=True,
                        )
                        nc.vector.tensor_copy(
                            o_stage[:, nch * (D // 2) : (nch + 1) * (D // 2)], po[:]
                        )
                    nc.sync.dma_start(out_d[st * P : (st + 1) * P, :], o_stage[:])

                # prologue: stage group 0 fully, prefetch group 1's x
                dma_group(0)
                for _ in phaseAB(0):
                    pass

                for t in range(NQT):
                    # pieces to interleave into this tile's kb loop: phase A/B
                    # of tile t+1 first, then out-projections of tile t-1
                    pieces = []
                    if t > 0:
                        pieces += [("op", (t - 1) * 4 + i) for i in range(4)]
                    gen = phaseAB(t + 1) if t + 1 < NQT else None

                    def next_piece(allow_op=True):
                        nonlocal gen
                        if gen is not None:
                            try:
                                next(gen)
                                return True
                            except StopIteration:
                                gen = None
                        if pieces and allow_op:
                            kind, v = pieces.pop(0)
                            outproj(v)
                            return True
                        return False

                    ctx = ctxp.tile([HD + 1, 2, QT_W], F32, name="ctx", tag="c")
                    nkb = 4 * (t + 1)
                    for kb in range(nkb):
                        r = kb * P - t * QT_W  # diagonal offset
                        r0 = max(0, r)
                        sc = work.tile([P, 2, QT_W], F32, name="sc", tag="w")
                        # K=64 row-tiled: slot 0 in array rows 0-63, slot 1 in
                        # rows 64-127 -> the two matmuls run concurrently
                        for slot in (0, 1):
                            nc.tensor.matmul(
                                sc[:, slot, r0:QT_W],
                                k2[
                                    slot * HD : (slot + 1) * HD,
                                    kb * P : (kb + 1) * P,
                                ],
                                qT[
                                    slot * HD : (slot + 1) * HD,
                                    t * QT_W + r0 : (t + 1) * QT_W,
                                ],
                                start=True,
                                stop=True,
                            )
                        if r >= 0:
                            nc.vector.tensor_tensor(
                                sc[:, :, r : r + P],
                                sc[:, :, r : r + P],
                                mask3[:, 0:1, :].broadcast_to([P, 2, P]),
                                ALU.add,
                            )
                        p2 = pt.tile([P, 2, QT_W], BF16, name="p2", tag="p")
                        nc.scalar.activation(
                            p2[:, :, r0:QT_W],
                            sc[:, :, r0:QT_W],
                            AF.Exp,
                            scale=0.125,
                        )
                        for slot in (0, 1):
                            nc.tensor.matmul(
                                ctx[:, slot, r0:QT_W],
                                vA[:, kb, slot, :],
                                p2[:, slot, r0:QT_W],
                                start=(kb == 0),
                                stop=(kb == nkb - 1),
                            )
                        next_piece(allow_op=(kb >= 10))

                    # ---- normalize into cT ----
                    # Ordering is delicate (DVE executes in program order):
                    # 1. DVE copy drains the ctx PSUM banks immediately (the
                    #    next tile's first ctx matmul waits on these banks)
                    # 2. leftover pieces drain next, so phaseAB(t+1)'s qT/k2
                    #    casts don't queue behind the normalize chain (else
                    #    the next tile's scores stall ~5us and HAM throttles)
                    # 3. only then the chunked reciprocal / bcast / multiply
                    ctx_sb = sm.tile(
                        [HD + 1, 2, QT_W], F32, name="ctx_sb", tag="csb", bufs=2
                    )
                    nc.vector.tensor_copy(ctx_sb[:], ctx[:])

                    while next_piece():
                        pass

                    lr = sm.tile([1, 2, QT_W], F32, name="lrecip", tag="lr")
                    RC = 8
                    for rc in range(RC):
                        w0, w1 = rc * (QT_W // RC), (rc + 1) * (QT_W // RC)
                        nc.vector.reciprocal(
                            lr[:, :, w0:w1], ctx_sb[HD : HD + 1, :, w0:w1]
                        )
                    lb = sm.tile([HD, 2, QT_W], F32, name="lb", tag="lb", bufs=2)
                    if t == NQT - 1:
                        # last tile: block-granular so each out-projection
                        # fires as soon as its 128-wide cT block is ready
                        for b in range(4):
                            nc.gpsimd.partition_broadcast(
                                lb[:, :, b * P : (b + 1) * P],
                                lr[0:1, :, b * P : (b + 1) * P],
                            )
                            for slot in (0, 1):
                                nc.vector.tensor_tensor(
                                    cT[
                                        slot * HD : (slot + 1) * HD,
                                        t * QT_W + b * P : t * QT_W + (b + 1) * P,
                                    ],
                                    ctx_sb[0:HD, slot, b * P : (b + 1) * P],
                                    lb[:, slot, b * P : (b + 1) * P],
                                    ALU.mult,
                                )
                            outproj(t * 4 + b)
                    else:
                        nc.gpsimd.partition_broadcast(lb[:], lr[0:1, :, :])
                        for slot in (0, 1):
                            nc.vector.tensor_tensor(
                                cT[
                                    slot * HD : (slot + 1) * HD,
                                    t * QT_W : (t + 1) * QT_W,
                                ],
                                ctx_sb[0:HD, slot, :],
                                lb[:, slot, :],
                                ALU.mult,
                            )

                if dbg:
                    nc.sync.dma_start(qT_dd[:], qT[:])
                    nc.sync.dma_start(kT_dd[:], k2[:])
                    nc.sync.dma_start(vA_dd[:], vA[:])
                    nc.sync.dma_start(cT_dd[:], cT[:])

    nc.compile()
    return nc


def _host_inputs(x, W_query, W_key, W_value, W_out):
    mask = np.where(
        np.arange(P)[:, None] <= np.arange(P)[None, :], 0.0, NEG
    ).astype(np.float32)
    ident = np.eye(P, dtype=ml_dtypes.bfloat16)
    xT_bf = np.ascontiguousarray(x.T.astype(ml_dtypes.bfloat16))
    in_maps = []
    for core in range(8):
        ha, hb = SLOTS[core]
        sa, sb = SCALES[core]
        ca, cb = slice(ha * HD, (ha + 1) * HD), slice(hb * HD, (hb + 1) * HD)
        w_all = np.concatenate(
            [
                W_query[:, ca],
                W_query[:, cb],
                W_key[:, ca],
                W_key[:, cb],
                W_value[:, ca],
                W_value[:, cb],
            ],
            axis=1,
        )
        in_maps.append(
            {
                "xT": xT_bf,
                "w": np.ascontiguousarray(w_all.astype(ml_dtypes.bfloat16)),
                "wo": np.ascontiguousarray(
                    np.concatenate([W_out[ca, :] * sa, W_out[cb, :] * sb], axis=0)
                ).astype(np.float32),
                "mask": mask,
                "ident": ident,
            }
        )
    return in_maps


def run(x, W_query, W_key, W_value, W_out, b_out, trace=False):
    global _CACHED_NC
    if _CACHED_NC is None:
        _CACHED_NC = build_nc()
    nc = _CACHED_NC
    in_maps = _host_inputs(x, W_query, W_key, W_value, W_out)
    res = run_bass_kernel_spmd(nc, in_maps, core_ids=list(range(8)), trace=trace)
    out = np.zeros((S, D), dtype=np.float32)
    for core in range(8):
        out += res.results[core]["out"].astype(np.float32)
    out += b_out[None, :].astype(np.float32)
    return out, res


def kernel(x, W_query, W_key, W_value, W_out, b_out):
    x2 = np.asarray(x, dtype=np.float32).reshape(S, D)
    out, _ = run(
        x2,
        np.asarray(W_query, np.float32),
        np.asarray(W_key, np.float32),
        np.asarray(W_value, np.float32),
        np.asarray(W_out, np.float32),
        np.asarray(b_out, np.float32),
    )
    return out.reshape(1, S, D)


# revision 19
# speedup vs baseline: 1.2334x; 1.2275x over previous
"""Causal multi-head attention (B=1, S=4096, D=768, H=12, d_head=64) on 8
Trainium2 NeuronCores.

Sharding: tensor-parallel over heads. 12 heads are mapped onto 16 head-slots
(2 per core); the 4 leftover heads are duplicated onto two slots of the same
core with their W_out rows pre-scaled by 0.5, keeping the SPMD program
uniform across cores. The host sums the 8 partial outputs and adds b_out
(the all-reduce step of the row-parallel out projection).

v4 (from v3):
- x is pre-transposed on the host (xT [D, S]); the on-device transpose
  pipeline (192 PE transposes at transpose-mode half clock + DVE repacks)
  is gone. Projections consume xT chunks directly.
- Score matmuls are K=64 row-tiled: slot 0 lives in array rows 0-63,
  slot 1 in rows 64-127 (tile_position auto-derived from base partitions),
  so the two slots' score matmuls run concurrently -> ~2x on scores.
- Normalize chain off the ACT queue: DVE drains ctx PSUM (was scalar.copy
  stuck behind the exp backlog -> ~5us PE stall per tile), denominator
  reciprocal via reciprocal_approx_fast (5x faster than reciprocal).
- Output partials in fp16 (halves output DMA, host accumulates in fp32).
"""

import sys

sys.path.insert(0, "/opt/trn_rl_repo")

import ml_dtypes
import numpy as np

import concourse.bass as bass
import concourse.tile as tile
from concourse import bacc, mybir
from concourse.bass_utils import run_bass_kernel_spmd

S = 4096
D = 768
HD = 64
P = 128
KC = D // P  # 6 contraction chunks for the projections
QT_W = 512  # query-tile width (psum free dim)
NQT = S // QT_W  # 8 query tiles
NKB = S // P  # 32 key blocks
NEG = -1e30

F32 = mybir.dt.float32
F32R = mybir.dt.float32r
F16 = mybir.dt.float16
BF16 = mybir.dt.bfloat16
AF = mybir.ActivationFunctionType
ALU = mybir.AluOpType

SLOTS = [(0, 1), (2, 3), (4, 5), (6, 7), (8, 8), (9, 9), (10, 10), (11, 11)]
SCALES = [(1.0, 1.0)] * 4 + [(0.5, 0.5)] * 4

_CACHED_NC = None


def build_nc(dbg=False):
    nc = bacc.Bacc("TRN2", target_bir_lowering=False, debug=False, num_devices=8)

    if dbg:
        qT_dd = nc.declare_dram_parameter("qT_dbg", [P, S], BF16, isOutput=True)
        kT_dd = nc.declare_dram_parameter("kT_dbg", [P, S], BF16, isOutput=True)
        vA_dd = nc.declare_dram_parameter(
            "vA_dbg", [P, NKB, 2, HD + 1], BF16, isOutput=True
        )
        cT_dd = nc.declare_dram_parameter("cT_dbg", [P, S], BF16, isOutput=True)

    xT_d = nc.declare_dram_parameter("xT", [D, S], BF16, isOutput=False)
    w_d = nc.declare_dram_parameter("w", [D, 3 * P], BF16, isOutput=False)
    wo_d = nc.declare_dram_parameter("wo", [P, D], F32, isOutput=False)
    mask_d = nc.declare_dram_parameter("mask", [P, P], F32, isOutput=False)
    ident_d = nc.declare_dram_parameter("ident", [P, P], BF16, isOutput=False)
    out_d = nc.declare_dram_parameter("out", [S, D], F16, isOutput=True)

    with tile.TileContext(nc) as tc:
        with (
            tc.tile_pool(name="const", bufs=1) as const,
            tc.tile_pool(name="big", bufs=1) as big,
        ):
            # ---- constants ----
            mask3 = const.tile([P, 1, P], F32)
            nc.sync.dma_start(mask3[:, 0, :], mask_d[:])
            ident = const.tile([P, P], BF16)
            nc.sync.dma_start(ident[:], ident_d[:])
            ones_c = const.tile([P, 1], BF16)
            nc.gpsimd.memset(ones_c[:], 1.0)
            w_r = const.tile([P, KC, 3 * P], BF16)
            nc.sync.dma_start(w_r[:], w_d.rearrange("(c p) m -> p c m", p=P))
            wo_r = const.tile([P, D], BF16)
            with tc.tile_pool(name="wst", bufs=1) as wst:
                wo_stage = wst.tile([P, D], F32)
                nc.sync.dma_start(wo_stage[:], wo_d[:])
                nc.vector.tensor_copy(wo_r[:], wo_stage[:])

            # Q^T / K^T with slot s in partitions [64s, 64s+64); V natural
            # [keys, slot, 65] with a ones column at 64 for the softmax
            # denominator.
            qT = big.tile([P, S], BF16)
            k2 = big.tile([P, S], BF16)
            vA = big.tile([P, NKB, 2, HD + 1], BF16)
            cT = big.tile([P, S], BF16)
            for slot in (0, 1):
                nc.vector.tensor_copy(
                    vA[:, :, slot, HD], ones_c[:, 0:1].broadcast_to([P, NKB])
                )

            xT_r = xT_d.rearrange("(c p) s -> p c s", p=P)

            with (
                tc.tile_pool(name="xs", bufs=3) as xs,
                tc.tile_pool(name="aux", bufs=2, space="PSUM") as aux,
                tc.tile_pool(name="work", bufs=2, space="PSUM") as work,
                tc.tile_pool(name="ctxp", bufs=1, space="PSUM") as ctxp,
                tc.tile_pool(name="sm", bufs=4) as sm,
                tc.tile_pool(name="pt", bufs=4) as pt,
            ):
                # warmup matmuls: ramp the PE HAM toward 2.4 GHz while the
                # first DMAs stream in (real matmuls - transpose-mode does
                # not count as PE-busy for the HAM)
                for wi in range(6):
                    wt = aux.tile([P, 4, P], F32, name="wt", tag="aux")
                    for c in range(4):
                        nc.tensor.matmul(
                            wt[:, c, :], ident[:], ident[:], start=True, stop=True
                        )

                stage_map = {}

                def dma_group(t):
                    xTt = xs.tile([P, KC, QT_W], BF16, name="xTt", tag="xs")
                    nc.sync.dma_start(
                        xTt[:], xT_r[:, :, t * QT_W : (t + 1) * QT_W]
                    )
                    stage_map[t] = xTt

                def phaseAB(t):
                    """Q/K/V projections for group t (xT already
                    DMA-prefetched), as an interleavable generator."""
                    xTt = stage_map.pop(t)
                    if t + 1 < NQT and t + 1 not in stage_map:
                        dma_group(t + 1)
                    yield
                    pq = aux.tile([P, QT_W], F32, name="pq", tag="aux")
                    for c in range(KC):
                        nc.tensor.matmul(
                            pq[:],
                            w_r[:, c, 0:P],
                            xTt[:, c, :],
                            start=(c == 0),
                            stop=(c == KC - 1),
                        )
                    yield
                    nc.vector.tensor_copy(qT[:, t * QT_W : (t + 1) * QT_W], pq[:])
                    pk = aux.tile([P, QT_W], F32, name="pk", tag="aux")
                    for c in range(KC):
                        nc.tensor.matmul(
                            pk[:],
                            w_r[:, c, P : 2 * P],
                            xTt[:, c, :],
                            start=(c == 0),
                            stop=(c == KC - 1),
                        )
                    yield
                    nc.vector.tensor_copy(k2[:, t * QT_W : (t + 1) * QT_W], pk[:])
                    pv = aux.tile([P, QT_W], F32, name="pv", tag="aux")
                    for c in range(KC):
                        nc.tensor.matmul(
                            pv[:],
                            w_r[:, c, 2 * P : 3 * P],
                            xTt[:, c, :],
                            start=(c == 0),
                            stop=(c == KC - 1),
                        )
                    yield
                    vt_t = sm.tile([P, QT_W], BF16, name="vt_t", tag="vt", bufs=2)
                    nc.vector.tensor_copy(vt_t[:], pv[:])
                    vp = aux.tile([P, 4, P], BF16, name="vp", tag="aux")
                    for b in range(4):
                        nc.tensor.transpose(
                            vp[:, b, :], vt_t[:, b * P : (b + 1) * P], ident[:]
                        )
                    yield
                    nc.vector.tensor_copy(
                        vA[:, t * 4 : (t + 1) * 4, :, 0:HD],
                        vp[:].rearrange("p b (s h) -> p b s h", s=2),
                    )
                    yield

                def outproj(st):
                    o_stage = sm.tile([P, D], F16, name="o_stage", tag="ost", bufs=3)
                    for nch in range(2):
                        po = aux.tile([P, D // 2], F32, name="po", tag="aux")
                        nc.tensor.matmul(
                            po[:],
                            cT[:, st * P : (st + 1) * P],
                            wo_r[:, nch * (D // 2) : (nch + 1) * (D // 2)],
                            start=True,
                            stop=True,
                        )
                        nc.vector.tensor_copy(
                            o_stage[:, nch * (D // 2) : (nch + 1) * (D // 2)], po[:]
                        )
                    nc.sync.dma_start(out_d[st * P : (st + 1) * P, :], o_stage[:])

                # prologue: stage group 0 fully, prefetch group 1's x
                dma_group(0)
                for _ in phaseAB(0):
                    pass

                for t in range(NQT):
                    # pieces to interleave into this tile's kb loop: phase A/B
                    # of tile t+1 first, then out-projections of tile t-1
                    pieces = []
                    if t > 0:
                        pieces += [("op", (t - 1) * 4 + i) for i in range(4)]
                    gen = phaseAB(t + 1) if t + 1 < NQT else None

                    def next_piece(allow_op=True):
                        nonlocal gen
                        if gen is not None:
                            try:
                                next(gen)
                                return True
                            except StopIteration:
                                gen = None
                        if pieces and allow_op:
                            kind, v = pieces.pop(0)
                            outproj(v)
                            return True
                        return False

                    ctx = ctxp.tile([HD + 1, 2, QT_W], F32, name="ctx", tag="c")
                    nkb = 4 * (t + 1)
                    for kb in range(nkb):
                        r = kb * P - t * QT_W  # diagonal offset
                        r0 = max(0, r)
                        sc = work.tile([P, 2, QT_W], F32, name="sc", tag="w")
                        # K=64 row-tiled: slot 0 in array rows 0-63, slot 1 in
                        # rows 64-127 -> the two matmuls run concurrently
                        for slot in (0, 1):
                            nc.tensor.matmul(
                                sc[:, slot, r0:QT_W],
                                k2[
                                    slot * HD : (slot + 1) * HD,
                                    kb * P : (kb + 1) * P,
                                ],
                                qT[
                                    slot * HD : (slot + 1) * HD,
                                    t * QT_W + r0 : (t + 1) * QT_W,
                                ],
                                start=True,
                                stop=True,
                            )
                        if r >= 0:
                            nc.vector.tensor_tensor(
                                sc[:, :, r : r + P],
                                sc[:, :, r : r + P],
                                mask3[:, 0:1, :].broadcast_to([P, 2, P]),
                                ALU.add,
                            )
                        p2 = pt.tile([P, 2, QT_W], BF16, name="p2", tag="p")
                        nc.scalar.activation(
                            p2[:, :, r0:QT_W],
                            sc[:, :, r0:QT_W],
                            AF.Exp,
                            scale=0.125,
                        )
                        for slot in (0, 1):
                            nc.tensor.matmul(
                                ctx[:, slot, r0:QT_W],
                                vA[:, kb, slot, :],
                                p2[:, slot, r0:QT_W],
                                start=(kb == 0),
                                stop=(kb == nkb - 1),
                            )
                        next_piece(allow_op=(kb >= 13))

                    # ---- normalize into cT ----
                    # Ordering is delicate (DVE executes in program order):
                    # 1. DMA scatters the denominator row (ctx PSUM row 64)
                    #    across 8 partitions, so the reciprocal runs on 8
                    #    lanes (~0.6us) instead of one (~7.7us serial DVE,
                    #    which pushed cT past the out-projection's LDW and
                    #    stalled the PE FIFO at every tile boundary)
                    # 2. DVE copy drains the ctx PSUM banks (the next tile's
                    #    first ctx matmul waits on these banks)
                    # 3. leftover pieces drain next, so phaseAB(t+1)'s qT/k2
                    #    casts don't queue behind the normalize chain
                    ctx_sb = sm.tile(
                        [HD + 1, 2, QT_W], F32, name="ctx_sb", tag="csb", bufs=2
                    )
                    nc.vector.tensor_copy(ctx_sb[:], ctx[:])
                    den8 = sm.tile([8, 2, QT_W // 8], F32, name="den8", tag="d8")
                    for slot in (0, 1):
                        nc.sync.dma_start(
                            den8[:, slot, :],
                            ctx_sb[HD : HD + 1, slot, :].rearrange(
                                "o (p w) -> o p w", p=8
                            ),
                        )

                    while next_piece():
                        pass

                    r8 = sm.tile([8, 2, QT_W // 8], F32, name="r8", tag="r8")
                    nc.vector.reciprocal(r8[:], den8[:])
                    lr = sm.tile([1, 2, QT_W], F32, name="lrecip", tag="lr")
                    for slot in (0, 1):
                        nc.sync.dma_start(
                            lr[:, slot, :].rearrange("o (p w) -> o p w", p=8),
                            r8[:, slot, :],
                        )
                    lb = sm.tile([HD, 2, QT_W], F32, name="lb", tag="lb", bufs=2)
                    if t == NQT - 1:
                        # last tile: block-granular so each out-projection
                        # fires as soon as its 128-wide cT block is ready
                        for b in range(4):
                            nc.gpsimd.partition_broadcast(
                                lb[:, :, b * P : (b + 1) * P],
                                lr[0:1, :, b * P : (b + 1) * P],
                            )
                            for slot in (0, 1):
                                nc.vector.tensor_tensor(
                                    cT[
                                        slot * HD : (slot + 1) * HD,
                                        t * QT_W + b * P : t * QT_W + (b + 1) * P,
                                    ],
                                    ctx_sb[0:HD, slot, b * P : (b + 1) * P],
                                    lb[:, slot, b * P : (b + 1) * P],
                                    ALU.mult,
                                )
                            outproj(t * 4 + b)
                    else:
                        nc.gpsimd.partition_broadcast(lb[:], lr[0:1, :, :])
                        for slot in (0, 1):
                            nc.vector.tensor_tensor(
                                cT[
                                    slot * HD : (slot + 1) * HD,
                                    t * QT_W : (t + 1) * QT_W,
                                ],
                                ctx_sb[0:HD, slot, :],
                                lb[:, slot, :],
                                ALU.mult,
                            )

                if dbg:
                    nc.sync.dma_start(qT_dd[:], qT[:])
                    nc.sync.dma_start(kT_dd[:], k2[:])
                    nc.sync.dma_start(vA_dd[:], vA[:])
                    nc.sync.dma_start(cT_dd[:], cT[:])

    nc.compile()
    return nc


def _host_inputs(x, W_query, W_key, W_value, W_out):
    mask = np.where(
        np.arange(P)[:, None] <= np.arange(P)[None, :], 0.0, NEG
    ).astype(np.float32)
    ident = np.eye(P, dtype=ml_dtypes.bfloat16)
    xT_bf = np.ascontiguousarray(x.T.astype(ml_dtypes.bfloat16))
    in_maps = []
    for core in range(8):
        ha, hb = SLOTS[core]
        sa, sb = SCALES[core]
        ca, cb = slice(ha * HD, (ha + 1) * HD), slice(hb * HD, (hb + 1) * HD)
        w_all = np.concatenate(
            [
                W_query[:, ca],
                W_query[:, cb],
                W_key[:, ca],
                W_key[:, cb],
                W_value[:, ca],
                W_value[:, cb],
            ],
            axis=1,
        )
        in_maps.append(
            {
                "xT": xT_bf,
                "w": np.ascontiguousarray(w_all.astype(ml_dtypes.bfloat16)),
                "wo": np.ascontiguousarray(
                    np.concatenate([W_out[ca, :] * sa, W_out[cb, :] * sb], axis=0)
                ).astype(np.float32),
                "mask": mask,
                "ident": ident,
            }
        )
    return in_maps


def run(x, W_query, W_key, W_value, W_out, b_out, trace=False):
    global _CACHED_NC
    if _CACHED_NC is None:
        _CACHED_NC = build_nc()
    nc = _CACHED_NC
    in_maps = _host_inputs(x, W_query, W_key, W_value, W_out)
    res = run_bass_kernel_spmd(nc, in_maps, core_ids=list(range(8)), trace=trace)
    out = np.zeros((S, D), dtype=np.float32)
    for core in range(8):
        out += res.results[core]["out"].astype(np.float32)
    out += b_out[None, :].astype(np.float32)
    return out, res


def kernel(x, W_query, W_key, W_value, W_out, b_out):
    x2 = np.asarray(x, dtype=np.float32).reshape(S, D)
    out, _ = run(
        x2,
        np.asarray(W_query, np.float32),
        np.asarray(W_key, np.float32),
        np.asarray(W_value, np.float32),
        np.asarray(W_out, np.float32),
        np.asarray(b_out, np.float32),
    )
    return out.reshape(1, S, D)


# revision 20
# speedup vs baseline: 1.2385x; 1.0041x over previous
"""Causal multi-head attention (B=1, S=4096, D=768, H=12, d_head=64) on 8
Trainium2 NeuronCores.

Sharding: tensor-parallel over heads. 12 heads are mapped onto 16 head-slots
(2 per core); the 4 leftover heads are duplicated onto two slots of the same
core with their W_out rows pre-scaled by 0.5, keeping the SPMD program
uniform across cores. The host sums the 8 partial outputs and adds b_out
(the all-reduce step of the row-parallel out projection).

v4 (from v3):
- x is pre-transposed on the host (xT [D, S]); the on-device transpose
  pipeline (192 PE transposes at transpose-mode half clock + DVE repacks)
  is gone. Projections consume xT chunks directly.
- Score matmuls are K=64 row-tiled: slot 0 lives in array rows 0-63,
  slot 1 in rows 64-127 (tile_position auto-derived from base partitions),
  so the two slots' score matmuls run concurrently -> ~2x on scores.
- Normalize chain off the ACT queue: DVE drains ctx PSUM (was scalar.copy
  stuck behind the exp backlog -> ~5us PE stall per tile), denominator
  reciprocal via reciprocal_approx_fast (5x faster than reciprocal).
- Output partials in fp16 (halves output DMA, host accumulates in fp32).
"""

import sys

sys.path.insert(0, "/opt/trn_rl_repo")

import ml_dtypes
import numpy as np

import concourse.bass as bass
import concourse.tile as tile
from concourse import bacc, mybir
from concourse.bass_utils import run_bass_kernel_spmd

S = 4096
D = 768
HD = 64
P = 128
KC = D // P  # 6 contraction chunks for the projections
QT_W = 512  # query-tile width (psum free dim)
NQT = S // QT_W  # 8 query tiles
NKB = S // P  # 32 key blocks
NEG = -1e30

F32 = mybir.dt.float32
F32R = mybir.dt.float32r
F16 = mybir.dt.float16
BF16 = mybir.dt.bfloat16
AF = mybir.ActivationFunctionType
ALU = mybir.AluOpType

SLOTS = [(0, 1), (2, 3), (4, 5), (6, 7), (8, 8), (9, 9), (10, 10), (11, 11)]
SCALES = [(1.0, 1.0)] * 4 + [(0.5, 0.5)] * 4

_CACHED_NC = None


def build_nc(dbg=False):
    nc = bacc.Bacc("TRN2", target_bir_lowering=False, debug=False, num_devices=8)

    if dbg:
        qT_dd = nc.declare_dram_parameter("qT_dbg", [P, S], BF16, isOutput=True)
        kT_dd = nc.declare_dram_parameter("kT_dbg", [P, S], BF16, isOutput=True)
        vA_dd = nc.declare_dram_parameter(
            "vA_dbg", [P, NKB, 2, HD + 1], BF16, isOutput=True
        )
        cT_dd = nc.declare_dram_parameter("cT_dbg", [P, S], BF16, isOutput=True)

    xT_d = nc.declare_dram_parameter("xT", [D, S], BF16, isOutput=False)
    w_d = nc.declare_dram_parameter("w", [D, 3 * P], BF16, isOutput=False)
    wo_d = nc.declare_dram_parameter("wo", [P, D], BF16, isOutput=False)
    mask_d = nc.declare_dram_parameter("mask", [P, P], F32, isOutput=False)
    ident_d = nc.declare_dram_parameter("ident", [P, P], BF16, isOutput=False)
    out_d = nc.declare_dram_parameter("out", [S, D], F16, isOutput=True)

    with tile.TileContext(nc) as tc:
        with (
            tc.tile_pool(name="const", bufs=1) as const,
            tc.tile_pool(name="big", bufs=1) as big,
        ):
            # ---- constants (tiles only; DMAs issued inside the inner
            # block so xT group 0 goes first on the sync queue) ----
            mask3 = const.tile([P, 1, P], F32)
            ident = const.tile([P, P], BF16)
            ones_c = const.tile([P, 1], BF16)
            zt = const.tile([P, P], BF16)
            w_r = const.tile([P, KC, 3 * P], BF16)
            wo_r = const.tile([P, D], BF16)

            # Q^T / K^T with slot s in partitions [64s, 64s+64); V natural
            # [keys, slot, 65] with a ones column at 64 for the softmax
            # denominator.
            qT = big.tile([P, S], BF16)
            k2 = big.tile([P, S], BF16)
            vA = big.tile([P, NKB, 2, HD + 1], BF16)
            cT = big.tile([P, S], BF16)

            xT_r = xT_d.rearrange("(c p) s -> p c s", p=P)

            with (
                tc.tile_pool(name="xs", bufs=3) as xs,
                tc.tile_pool(name="aux", bufs=2, space="PSUM") as aux,
                tc.tile_pool(name="work", bufs=2, space="PSUM") as work,
                tc.tile_pool(name="ctxp", bufs=1, space="PSUM") as ctxp,
                tc.tile_pool(name="sm", bufs=4) as sm,
                tc.tile_pool(name="pt", bufs=4) as pt,
            ):
                stage_map = {}

                def dma_group(t):
                    xTt = xs.tile([P, KC, QT_W], BF16, name="xTt", tag="xs")
                    nc.sync.dma_start(
                        xTt[:], xT_r[:, :, t * QT_W : (t + 1) * QT_W]
                    )
                    stage_map[t] = xTt

                # xT group 0 + weights first on the DMA queue (the first
                # projection waits on them), then the small constants
                dma_group(0)
                nc.sync.dma_start(w_r[:], w_d.rearrange("(c p) m -> p c m", p=P))
                nc.gpsimd.memset(zt[:], 0.0)
                nc.gpsimd.memset(ones_c[:], 1.0)
                nc.sync.dma_start(mask3[:, 0, :], mask_d[:])
                nc.sync.dma_start(ident[:], ident_d[:])
                nc.sync.dma_start(wo_r[:], wo_d[:])
                for slot in (0, 1):
                    nc.vector.tensor_copy(
                        vA[:, :, slot, HD], ones_c[:, 0:1].broadcast_to([P, NKB])
                    )

                # warmup matmuls: ramp the PE HAM toward 2.4 GHz while the
                # first DMAs stream in (zero tile - no DMA dependency, and
                # real matmuls: transpose-mode does not count for the HAM)
                for wi in range(6):
                    wt = aux.tile([P, 4, P], F32, name="wt", tag="aux")
                    for c in range(4):
                        nc.tensor.matmul(
                            wt[:, c, :], zt[:], zt[:], start=True, stop=True
                        )

                def phaseAB(t):
                    """Q/K/V projections for group t (xT already
                    DMA-prefetched), as an interleavable generator."""
                    xTt = stage_map.pop(t)
                    if t + 1 < NQT and t + 1 not in stage_map:
                        dma_group(t + 1)
                    yield
                    pq = aux.tile([P, QT_W], F32, name="pq", tag="aux")
                    for c in range(KC):
                        nc.tensor.matmul(
                            pq[:],
                            w_r[:, c, 0:P],
                            xTt[:, c, :],
                            start=(c == 0),
                            stop=(c == KC - 1),
                        )
                    yield
                    nc.vector.tensor_copy(qT[:, t * QT_W : (t + 1) * QT_W], pq[:])
                    pk = aux.tile([P, QT_W], F32, name="pk", tag="aux")
                    for c in range(KC):
                        nc.tensor.matmul(
                            pk[:],
                            w_r[:, c, P : 2 * P],
                            xTt[:, c, :],
                            start=(c == 0),
                            stop=(c == KC - 1),
                        )
                    yield
                    nc.vector.tensor_copy(k2[:, t * QT_W : (t + 1) * QT_W], pk[:])
                    pv = aux.tile([P, QT_W], F32, name="pv", tag="aux")
                    for c in range(KC):
                        nc.tensor.matmul(
                            pv[:],
                            w_r[:, c, 2 * P : 3 * P],
                            xTt[:, c, :],
                            start=(c == 0),
                            stop=(c == KC - 1),
                        )
                    yield
                    vt_t = sm.tile([P, QT_W], BF16, name="vt_t", tag="vt", bufs=2)
                    nc.vector.tensor_copy(vt_t[:], pv[:])
                    vp = aux.tile([P, 4, P], BF16, name="vp", tag="aux")
                    for b in range(4):
                        nc.tensor.transpose(
                            vp[:, b, :], vt_t[:, b * P : (b + 1) * P], ident[:]
                        )
                    yield
                    nc.vector.tensor_copy(
                        vA[:, t * 4 : (t + 1) * 4, :, 0:HD],
                        vp[:].rearrange("p b (s h) -> p b s h", s=2),
                    )
                    yield

                def outproj(st):
                    o_stage = sm.tile([P, D], F16, name="o_stage", tag="ost", bufs=3)
                    for nch in range(2):
                        po = aux.tile([P, D // 2], F32, name="po", tag="aux")
                        nc.tensor.matmul(
                            po[:],
                            cT[:, st * P : (st + 1) * P],
                            wo_r[:, nch * (D // 2) : (nch + 1) * (D // 2)],
                            start=True,
                            stop=True,
                        )
                        nc.vector.tensor_copy(
                            o_stage[:, nch * (D // 2) : (nch + 1) * (D // 2)], po[:]
                        )
                    nc.sync.dma_start(out_d[st * P : (st + 1) * P, :], o_stage[:])

                # prologue: group 0 staged above; prefetch group 1's x
                for _ in phaseAB(0):
                    pass

                for t in range(NQT):
                    # pieces to interleave into this tile's kb loop: phase A/B
                    # of tile t+1 first, then out-projections of tile t-1
                    pieces = []
                    if t > 0:
                        pieces += [("op", (t - 1) * 4 + i) for i in range(4)]
                    gen = phaseAB(t + 1) if t + 1 < NQT else None

                    def next_piece(allow_op=True):
                        nonlocal gen
                        if gen is not None:
                            try:
                                next(gen)
                                return True
                            except StopIteration:
                                gen = None
                        if pieces and allow_op:
                            kind, v = pieces.pop(0)
                            outproj(v)
                            return True
                        return False

                    ctx = ctxp.tile([HD + 1, 2, QT_W], F32, name="ctx", tag="c")
                    nkb = 4 * (t + 1)
                    for kb in range(nkb):
                        r = kb * P - t * QT_W  # diagonal offset
                        r0 = max(0, r)
                        sc = work.tile([P, 2, QT_W], F32, name="sc", tag="w")
                        # K=64 row-tiled: slot 0 in array rows 0-63, slot 1 in
                        # rows 64-127 -> the two matmuls run concurrently
                        for slot in (0, 1):
                            nc.tensor.matmul(
                                sc[:, slot, r0:QT_W],
                                k2[
                                    slot * HD : (slot + 1) * HD,
                                    kb * P : (kb + 1) * P,
                                ],
                                qT[
                                    slot * HD : (slot + 1) * HD,
                                    t * QT_W + r0 : (t + 1) * QT_W,
                                ],
                                start=True,
                                stop=True,
                            )
                        if r >= 0:
                            nc.vector.tensor_tensor(
                                sc[:, :, r : r + P],
                                sc[:, :, r : r + P],
                                mask3[:, 0:1, :].broadcast_to([P, 2, P]),
                                ALU.add,
                            )
                        p2 = pt.tile([P, 2, QT_W], BF16, name="p2", tag="p")
                        nc.scalar.activation(
                            p2[:, :, r0:QT_W],
                            sc[:, :, r0:QT_W],
                            AF.Exp,
                            scale=0.125,
                        )
                        for slot in (0, 1):
                            nc.tensor.matmul(
                                ctx[:, slot, r0:QT_W],
                                vA[:, kb, slot, :],
                                p2[:, slot, r0:QT_W],
                                start=(kb == 0),
                                stop=(kb == nkb - 1),
                            )
                        next_piece(allow_op=(kb >= 13))

                    # ---- normalize into cT ----
                    # Ordering is delicate (DVE executes in program order):
                    # 1. DMA scatters the denominator row (ctx PSUM row 64)
                    #    across 8 partitions, so the reciprocal runs on 8
                    #    lanes (~0.6us) instead of one (~7.7us serial DVE,
                    #    which pushed cT past the out-projection's LDW and
                    #    stalled the PE FIFO at every tile boundary)
                    # 2. DVE copy drains the ctx PSUM banks (the next tile's
                    #    first ctx matmul waits on these banks)
                    # 3. leftover pieces drain next, so phaseAB(t+1)'s qT/k2
                    #    casts don't queue behind the normalize chain
                    ctx_sb = sm.tile(
                        [HD + 1, 2, QT_W], F32, name="ctx_sb", tag="csb", bufs=2
                    )
                    nc.vector.tensor_copy(ctx_sb[:], ctx[:])
                    den8 = sm.tile([8, 2, QT_W // 8], F32, name="den8", tag="d8")
                    for slot in (0, 1):
                        nc.gpsimd.dma_start(
                            den8[:, slot, :],
                            ctx_sb[HD : HD + 1, slot, :].rearrange(
                                "o (p w) -> o p w", p=8
                            ),
                        )

                    while next_piece():
                        pass

                    r8 = sm.tile([8, 2, QT_W // 8], F32, name="r8", tag="r8")
                    nc.vector.reciprocal(r8[:], den8[:])
                    lr = sm.tile([1, 2, QT_W], F32, name="lrecip", tag="lr")
                    for slot in (0, 1):
                        nc.gpsimd.dma_start(
                            lr[:, slot, :].rearrange("o (p w) -> o p w", p=8),
                            r8[:, slot, :],
                        )
                    lb = sm.tile([HD, 2, QT_W], F32, name="lb", tag="lb", bufs=2)
                    if t == NQT - 1:
                        # last tile: block-granular so each out-projection
                        # fires as soon as its 128-wide cT block is ready
                        for b in range(4):
                            nc.gpsimd.partition_broadcast(
                                lb[:, :, b * P : (b + 1) * P],
                                lr[0:1, :, b * P : (b + 1) * P],
                            )
                            for slot in (0, 1):
                                nc.vector.tensor_tensor(
                                    cT[
                                        slot * HD : (slot + 1) * HD,
                                        t * QT_W + b * P : t * QT_W + (b + 1) * P,
                                    ],
                                    ctx_sb[0:HD, slot, b * P : (b + 1) * P],
                                    lb[:, slot, b * P : (b + 1) * P],
                                    ALU.mult,
                                )
                            outproj(t * 4 + b)
                    else:
                        nc.gpsimd.partition_broadcast(lb[:], lr[0:1, :, :])
                        for slot in (0, 1):
                            nc.vector.tensor_tensor(
                                cT[
                                    slot * HD : (slot + 1) * HD,
                                    t * QT_W : (t + 1) * QT_W,
                                ],
                                ctx_sb[0:HD, slot, :],
                                lb[:, slot, :],
                                ALU.mult,
                            )

                if dbg:
                    nc.sync.dma_start(qT_dd[:], qT[:])
                    nc.sync.dma_start(kT_dd[:], k2[:])
                    nc.sync.dma_start(vA_dd[:], vA[:])
                    nc.sync.dma_start(cT_dd[:], cT[:])

    nc.compile()
    return nc


def _host_inputs(x, W_query, W_key, W_value, W_out):
    mask = np.where(
        np.arange(P)[:, None] <= np.arange(P)[None, :], 0.0, NEG
    ).astype(np.float32)
    ident = np.eye(P, dtype=ml_dtypes.bfloat16)
    xT_bf = np.ascontiguousarray(x.T.astype(ml_dtypes.bfloat16))
    in_maps = []
    for core in range(8):
        ha, hb = SLOTS[core]
        sa, sb = SCALES[core]
        ca, cb = slice(ha * HD, (ha + 1) * HD), slice(hb * HD, (hb + 1) * HD)
        w_all = np.concatenate(
            [
                W_query[:, ca],
                W_query[:, cb],
                W_key[:, ca],
                W_key[:, cb],
                W_value[:, ca],
                W_value[:, cb],
            ],
            axis=1,
        )
        in_maps.append(
            {
                "xT": xT_bf,
                "w": np.ascontiguousarray(w_all.astype(ml_dtypes.bfloat16)),
                "wo": np.ascontiguousarray(
                    np.concatenate([W_out[ca, :] * sa, W_out[cb, :] * sb], axis=0)
                ).astype(ml_dtypes.bfloat16),
                "mask": mask,
                "ident": ident,
            }
        )
    return in_maps


def run(x, W_query, W_key, W_value, W_out, b_out, trace=False):
    global _CACHED_NC
    if _CACHED_NC is None:
        _CACHED_NC = build_nc()
    nc = _CACHED_NC
    in_maps = _host_inputs(x, W_query, W_key, W_value, W_out)
    res = run_bass_kernel_spmd(nc, in_maps, core_ids=list(range(8)), trace=trace)
    out = np.zeros((S, D), dtype=np.float32)
    for core in range(8):
        out += res.results[core]["out"].astype(np.float32)
    out += b_out[None, :].astype(np.float32)
    return out, res


def kernel(x, W_query, W_key, W_value, W_out, b_out):
    x2 = np.asarray(x, dtype=np.float32).reshape(S, D)
    out, _ = run(
        x2,
        np.asarray(W_query, np.float32),
        np.asarray(W_key, np.float32),
        np.asarray(W_value, np.float32),
        np.asarray(W_out, np.float32),
        np.asarray(b_out, np.float32),
    )
    return out.reshape(1, S, D)


# revision 22
# speedup vs baseline: 1.2858x; 1.0382x over previous
"""Causal multi-head attention (B=1, S=4096, D=768, H=12, d_head=64) on 8
Trainium2 NeuronCores.

Sharding: tensor-parallel over heads. 12 heads are mapped onto 16 head-slots
(2 per core); the 4 leftover heads are duplicated onto two slots of the same
core with their W_out rows pre-scaled by 0.5, keeping the SPMD program
uniform across cores. The host sums the 8 partial outputs and adds b_out
(the all-reduce step of the row-parallel out projection).

v4 (from v3):
- x is pre-transposed on the host (xT [D, S]); the on-device transpose
  pipeline (192 PE transposes at transpose-mode half clock + DVE repacks)
  is gone. Projections consume xT chunks directly.
- Score matmuls are K=64 row-tiled: slot 0 lives in array rows 0-63,
  slot 1 in rows 64-127 (tile_position auto-derived from base partitions),
  so the two slots' score matmuls run concurrently -> ~2x on scores.
- Normalize chain off the ACT queue: DVE drains ctx PSUM (was scalar.copy
  stuck behind the exp backlog -> ~5us PE stall per tile), denominator
  reciprocal via reciprocal_approx_fast (5x faster than reciprocal).
- Output partials in fp16 (halves output DMA, host accumulates in fp32).
"""

import sys

sys.path.insert(0, "/opt/trn_rl_repo")

import ml_dtypes
import numpy as np

import concourse.bass as bass
import concourse.tile as tile
from concourse import bacc, mybir
from concourse.bass_utils import run_bass_kernel_spmd

S = 4096
D = 768
HD = 64
P = 128
KC = D // P  # 6 contraction chunks for the projections
QT_W = 512  # query-tile width (psum free dim)
NQT = S // QT_W  # 8 query tiles
NKB = S // P  # 32 key blocks
NEG = -1e30

F32 = mybir.dt.float32
F32R = mybir.dt.float32r
F16 = mybir.dt.float16
BF16 = mybir.dt.bfloat16
AF = mybir.ActivationFunctionType
ALU = mybir.AluOpType

SLOTS = [(0, 1), (2, 3), (4, 5), (6, 7), (8, 8), (9, 9), (10, 10), (11, 11)]
SCALES = [(1.0, 1.0)] * 4 + [(0.5, 0.5)] * 4

_CACHED_NC = None


def build_nc(dbg=False):
    nc = bacc.Bacc("TRN2", target_bir_lowering=False, debug=False, num_devices=8)

    if dbg:
        qT_dd = nc.declare_dram_parameter("qT_dbg", [P, S], BF16, isOutput=True)
        kT_dd = nc.declare_dram_parameter("kT_dbg", [P, S], BF16, isOutput=True)
        vA_dd = nc.declare_dram_parameter(
            "vA_dbg", [P, NKB, 2, HD + 1], BF16, isOutput=True
        )
        cT_dd = nc.declare_dram_parameter("cT_dbg", [P, S], BF16, isOutput=True)

    xT_d = nc.declare_dram_parameter("xT", [D, S], BF16, isOutput=False)
    w_d = nc.declare_dram_parameter("w", [D, 3 * P], BF16, isOutput=False)
    wo_d = nc.declare_dram_parameter("wo", [P, D], BF16, isOutput=False)
    mask_d = nc.declare_dram_parameter("mask", [P, P], F32, isOutput=False)
    ident_d = nc.declare_dram_parameter("ident", [P, P], BF16, isOutput=False)
    out_d = nc.declare_dram_parameter("out", [S, D], F16, isOutput=True)

    with tile.TileContext(nc) as tc:
        with (
            tc.tile_pool(name="const", bufs=1) as const,
            tc.tile_pool(name="big", bufs=1) as big,
        ):
            # ---- constants (tiles only; DMAs issued inside the inner
            # block so xT group 0 goes first on the sync queue) ----
            mask3 = const.tile([P, 1, P], F32)
            ident = const.tile([P, P], BF16)
            ones_c = const.tile([P, 1], BF16)
            zt = const.tile([P, P], BF16)
            w_r = const.tile([P, KC, 3 * P], BF16)
            wo_r = const.tile([P, D], BF16)

            # Q^T / K^T with slot s in partitions [64s, 64s+64); V natural
            # [keys, slot, 65] with a ones column at 64 for the softmax
            # denominator.
            qT = big.tile([P, S], BF16)
            k2 = big.tile([P, S], BF16)
            vA = big.tile([P, NKB, 2, HD + 1], BF16)
            cT = big.tile([P, S], BF16)

            xT_r = xT_d.rearrange("(c p) s -> p c s", p=P)

            with (
                tc.tile_pool(name="xs", bufs=3) as xs,
                tc.tile_pool(name="aux", bufs=2, space="PSUM") as aux,
                tc.tile_pool(name="work", bufs=2, space="PSUM") as work,
                tc.tile_pool(name="ctxp", bufs=1, space="PSUM") as ctxp,
                tc.tile_pool(name="sm", bufs=4) as sm,
                tc.tile_pool(name="pt", bufs=4) as pt,
            ):
                stage_map = {}

                def dma_group(t):
                    xTt = xs.tile([P, KC, QT_W], BF16, name="xTt", tag="xs")
                    nc.sync.dma_start(
                        xTt[:], xT_r[:, :, t * QT_W : (t + 1) * QT_W]
                    )
                    stage_map[t] = xTt

                # xT group 0 + weights first on the DMA queue (the first
                # projection waits on them), then the small constants
                dma_group(0)
                nc.sync.dma_start(w_r[:], w_d.rearrange("(c p) m -> p c m", p=P))
                nc.gpsimd.memset(zt[:], 0.0)
                nc.gpsimd.memset(ones_c[:], 1.0)
                nc.sync.dma_start(mask3[:, 0, :], mask_d[:])
                nc.sync.dma_start(ident[:], ident_d[:])
                nc.sync.dma_start(wo_r[:], wo_d[:])
                for slot in (0, 1):
                    nc.vector.tensor_copy(
                        vA[:, :, slot, HD], ones_c[:, 0:1].broadcast_to([P, NKB])
                    )

                # warmup matmuls: ramp the PE HAM toward 2.4 GHz while the
                # first DMAs stream in (zero tile - no DMA dependency, and
                # real matmuls: transpose-mode does not count for the HAM)
                for wi in range(6):
                    wt = aux.tile([P, 4, P], F32, name="wt", tag="aux")
                    for c in range(4):
                        nc.tensor.matmul(
                            wt[:, c, :], zt[:], zt[:], start=True, stop=True
                        )

                def phaseAB(t):
                    """Q/K/V projections for group t (xT already
                    DMA-prefetched), as an interleavable generator."""
                    xTt = stage_map.pop(t)
                    if t + 1 < NQT and t + 1 not in stage_map:
                        dma_group(t + 1)
                    yield
                    pq = aux.tile([P, QT_W], F32, name="pq", tag="aux")
                    for c in range(KC):
                        nc.tensor.matmul(
                            pq[:],
                            w_r[:, c, 0:P],
                            xTt[:, c, :],
                            start=(c == 0),
                            stop=(c == KC - 1),
                        )
                    yield
                    nc.vector.tensor_copy(qT[:, t * QT_W : (t + 1) * QT_W], pq[:])
                    pk = aux.tile([P, QT_W], F32, name="pk", tag="aux")
                    for c in range(KC):
                        nc.tensor.matmul(
                            pk[:],
                            w_r[:, c, P : 2 * P],
                            xTt[:, c, :],
                            start=(c == 0),
                            stop=(c == KC - 1),
                        )
                    yield
                    nc.vector.tensor_copy(k2[:, t * QT_W : (t + 1) * QT_W], pk[:])
                    pv = aux.tile([P, QT_W], F32, name="pv", tag="aux")
                    for c in range(KC):
                        nc.tensor.matmul(
                            pv[:],
                            w_r[:, c, 2 * P : 3 * P],
                            xTt[:, c, :],
                            start=(c == 0),
                            stop=(c == KC - 1),
                        )
                    yield
                    vt_t = sm.tile([P, QT_W], BF16, name="vt_t", tag="vt", bufs=2)
                    nc.vector.tensor_copy(vt_t[:], pv[:])
                    vp = aux.tile([P, 4, P], BF16, name="vp", tag="aux")
                    for b in range(4):
                        nc.tensor.transpose(
                            vp[:, b, :], vt_t[:, b * P : (b + 1) * P], ident[:]
                        )
                    yield
                    nc.vector.tensor_copy(
                        vA[:, t * 4 : (t + 1) * 4, :, 0:HD],
                        vp[:].rearrange("p b (s h) -> p b s h", s=2),
                    )
                    yield

                def outproj(st):
                    o_stage = sm.tile([P, D], F16, name="o_stage", tag="ost", bufs=3)
                    for nch in range(2):
                        po = aux.tile([P, D // 2], F32, name="po", tag="aux")
                        nc.tensor.matmul(
                            po[:],
                            cT[:, st * P : (st + 1) * P],
                            wo_r[:, nch * (D // 2) : (nch + 1) * (D // 2)],
                            start=True,
                            stop=True,
                        )
                        nc.vector.tensor_copy(
                            o_stage[:, nch * (D // 2) : (nch + 1) * (D // 2)], po[:]
                        )
                    nc.sync.dma_start(out_d[st * P : (st + 1) * P, :], o_stage[:])

                # prologue: group 0 staged above; prefetch group 1's x
                for _ in phaseAB(0):
                    pass

                # out-projection blocks whose cT is finalized but which have
                # not been issued yet; consumed deep inside later kb loops so
                # their cT read never blocks the PE FIFO on the (slow-ish)
                # normalize chain
                pending_ops = []

                for t in range(NQT):
                    # pieces to interleave into this tile's kb loop: phase A/B
                    # of tile t+1 first, then pending out-projections
                    gen = phaseAB(t + 1) if t + 1 < NQT else None

                    def next_piece(allow_op=True):
                        nonlocal gen
                        if gen is not None:
                            try:
                                next(gen)
                                return True
                            except StopIteration:
                                gen = None
                        if pending_ops and allow_op:
                            outproj(pending_ops.pop(0))
                            return True
                        return False

                    ctx = ctxp.tile([HD + 1, 2, QT_W], F32, name="ctx", tag="c")
                    nkb = 4 * (t + 1)
                    for kb in range(nkb):
                        r = kb * P - t * QT_W  # diagonal offset
                        r0 = max(0, r)
                        sc = work.tile([P, 2, QT_W], F32, name="sc", tag="w")
                        # K=64 row-tiled: slot 0 in array rows 0-63, slot 1 in
                        # rows 64-127 -> the two matmuls run concurrently
                        for slot in (0, 1):
                            nc.tensor.matmul(
                                sc[:, slot, r0:QT_W],
                                k2[
                                    slot * HD : (slot + 1) * HD,
                                    kb * P : (kb + 1) * P,
                                ],
                                qT[
                                    slot * HD : (slot + 1) * HD,
                                    t * QT_W + r0 : (t + 1) * QT_W,
                                ],
                                start=True,
                                stop=True,
                            )
                        if r >= 0:
                            nc.vector.tensor_tensor(
                                sc[:, :, r : r + P],
                                sc[:, :, r : r + P],
                                mask3[:, 0:1, :].broadcast_to([P, 2, P]),
                                ALU.add,
                            )
                        p2 = pt.tile([P, 2, QT_W], BF16, name="p2", tag="p")
                        nc.scalar.activation(
                            p2[:, :, r0:QT_W],
                            sc[:, :, r0:QT_W],
                            AF.Exp,
                            scale=0.125,
                        )
                        for slot in (0, 1):
                            nc.tensor.matmul(
                                ctx[:, slot, r0:QT_W],
                                vA[:, kb, slot, :],
                                p2[:, slot, r0:QT_W],
                                start=(kb == 0),
                                stop=(kb == nkb - 1),
                            )
                        next_piece(allow_op=(kb >= 13))

                    # ---- normalize into cT ----
                    # Ordering is delicate (DVE executes in program order):
                    # 1. DMA scatters the denominator row (ctx PSUM row 64)
                    #    across 8 partitions, so the reciprocal runs on 8
                    #    lanes (~0.6us) instead of one (~7.7us serial DVE,
                    #    which pushed cT past the out-projection's LDW and
                    #    stalled the PE FIFO at every tile boundary)
                    # 2. DVE copy drains the ctx PSUM banks (the next tile's
                    #    first ctx matmul waits on these banks)
                    # 3. leftover pieces drain next, so phaseAB(t+1)'s qT/k2
                    #    casts don't queue behind the normalize chain
                    ctx_sb = sm.tile(
                        [HD + 1, 2, QT_W], F32, name="ctx_sb", tag="csb", bufs=2
                    )
                    nc.vector.tensor_copy(ctx_sb[:], ctx[:])
                    den8 = sm.tile([8, 2, QT_W // 8], F32, name="den8", tag="d8")
                    for slot in (0, 1):
                        nc.gpsimd.dma_start(
                            den8[:, slot, :],
                            ctx_sb[HD : HD + 1, slot, :].rearrange(
                                "o (p w) -> o p w", p=8
                            ),
                        )

                    while next_piece(allow_op=False):
                        pass

                    r8 = sm.tile([8, 2, QT_W // 8], F32, name="r8", tag="r8")
                    nc.vector.reciprocal(r8[:], den8[:])
                    lr = sm.tile([1, 2, QT_W], F32, name="lrecip", tag="lr")
                    for slot in (0, 1):
                        nc.gpsimd.dma_start(
                            lr[:, slot, :].rearrange("o (p w) -> o p w", p=8),
                            r8[:, slot, :],
                        )
                    lb = sm.tile([HD, 2, QT_W], F32, name="lb", tag="lb", bufs=2)
                    if t == NQT - 1:
                        # last tile: block-granular so each out-projection
                        # fires as soon as its 128-wide cT block is ready
                        for b in range(4):
                            nc.gpsimd.partition_broadcast(
                                lb[:, :, b * P : (b + 1) * P],
                                lr[0:1, :, b * P : (b + 1) * P],
                            )
                            for slot in (0, 1):
                                nc.vector.tensor_tensor(
                                    cT[
                                        slot * HD : (slot + 1) * HD,
                                        t * QT_W + b * P : t * QT_W + (b + 1) * P,
                                    ],
                                    ctx_sb[0:HD, slot, b * P : (b + 1) * P],
                                    lb[:, slot, b * P : (b + 1) * P],
                                    ALU.mult,
                                )
                            outproj(t * 4 + b)
                        while pending_ops:
                            outproj(pending_ops.pop(0))
                    else:
                        nc.gpsimd.partition_broadcast(lb[:], lr[0:1, :, :])
                        for slot in (0, 1):
                            nc.vector.tensor_tensor(
                                cT[
                                    slot * HD : (slot + 1) * HD,
                                    t * QT_W : (t + 1) * QT_W,
                                ],
                                ctx_sb[0:HD, slot, :],
                                lb[:, slot, :],
                                ALU.mult,
                            )
                        pending_ops.extend(t * 4 + i for i in range(4))

                if dbg:
                    nc.sync.dma_start(qT_dd[:], qT[:])
                    nc.sync.dma_start(kT_dd[:], k2[:])
                    nc.sync.dma_start(vA_dd[:], vA[:])
                    nc.sync.dma_start(cT_dd[:], cT[:])

    nc.compile()
    return nc


def _host_inputs(x, W_query, W_key, W_value, W_out):
    mask = np.where(
        np.arange(P)[:, None] <= np.arange(P)[None, :], 0.0, NEG
    ).astype(np.float32)
    ident = np.eye(P, dtype=ml_dtypes.bfloat16)
    xT_bf = np.ascontiguousarray(x.T.astype(ml_dtypes.bfloat16))
    in_maps = []
    for core in range(8):
        ha, hb = SLOTS[core]
        sa, sb = SCALES[core]
        ca, cb = slice(ha * HD, (ha + 1) * HD), slice(hb * HD, (hb + 1) * HD)
        w_all = np.concatenate(
            [
                W_query[:, ca],
                W_query[:, cb],
                W_key[:, ca],
                W_key[:, cb],
                W_value[:, ca],
                W_value[:, cb],
            ],
            axis=1,
        )
        in_maps.append(
            {
                "xT": xT_bf,
                "w": np.ascontiguousarray(w_all.astype(ml_dtypes.bfloat16)),
                "wo": np.ascontiguousarray(
                    np.concatenate([W_out[ca, :] * sa, W_out[cb, :] * sb], axis=0)
                ).astype(ml_dtypes.bfloat16),
                "mask": mask,
                "ident": ident,
            }
        )
    return in_maps


def run(x, W_query, W_key, W_value, W_out, b_out, trace=False):
    global _CACHED_NC
    if _CACHED_NC is None:
        _CACHED_NC = build_nc()
    nc = _CACHED_NC
    in_maps = _host_inputs(x, W_query, W_key, W_value, W_out)
    res = run_bass_kernel_spmd(nc, in_maps, core_ids=list(range(8)), trace=trace)
    out = np.zeros((S, D), dtype=np.float32)
    for core in range(8):
        out += res.results[core]["out"].astype(np.float32)
    out += b_out[None, :].astype(np.float32)
    return out, res


def kernel(x, W_query, W_key, W_value, W_out, b_out):
    x2 = np.asarray(x, dtype=np.float32).reshape(S, D)
    out, _ = run(
        x2,
        np.asarray(W_query, np.float32),
        np.asarray(W_key, np.float32),
        np.asarray(W_value, np.float32),
        np.asarray(W_out, np.float32),
        np.asarray(b_out, np.float32),
    )
    return out.reshape(1, S, D)
